# revision 1
# baseline (speedup 1.0000x reference)
"""MemTransformerLM (Transformer-XL) forward on 8 Trainium2 NeuronCores.

Sharding: data-parallel over batch (4) x tensor-parallel over heads/FFN/vocab
(2), Megatron-style, with 2 bf16 AllReduces per layer over 2-core groups.

Numerics: bf16 storage / fp32 PSUM accumulation; LayerNorm statistics in
fp32/fp32r. rel_shift is done exactly via a DRAM round-trip with mismatched
write/read row strides (write stride KLEN+QLEN-1, read stride KLEN+QLEN-2),
with the pad region pre-filled with -1e38 so causal masking comes for free.

v3: the per-layer AllReduces are split into two 256-token halves and
pipelined (residual+LN+FFN run per half as each AllReduce lands);
next layer's h-independent work (rkT, cond+mems part of k/v) is
emitted around the AllReduces via double-buffered kT/rkT/v_tok and
single-slot rotated weight tiles; the AllReduce readback accumulates
straight into the residual stream with a gpsimd accum-DMA; LayerNorm
stats are computed column-major so the reciprocal runs on 128 DVE lanes
instead of one; skew reads ride the idle ACT HWDGE ring; fully-masked
prob transpose segments are skipped (probT pre-zeroed once).

Self-contained: hardcodes all shapes; takes full inputs, returns full output.
"""
import os
import sys
import types

sys.path.insert(0, '/opt/trn_rl_repo')

import numpy as np


def _install_axon_ntff_shim():
    try:
        from antenv import axon_hooks  # noqa: F401
        return
    except ImportError:
        pass
    try:
        import antenv
        mod = types.ModuleType("antenv.axon_hooks")
        mod._hook = None

        def _set(h):
            mod._hook = h

        def _get():
            return mod._hook

        mod.set_axon_ntff_profile_hook = _set
        mod.get_axon_ntff_profile_hook = _get
        sys.modules["antenv.axon_hooks"] = mod
        antenv.axon_hooks = mod
        from trn_agent_boot.trn_boot import _ntff_profile_via_ctypes
        hook = _ntff_profile_via_ctypes('/opt/axon/libaxon_pjrt.so')
        if hook is not None:
            mod.set_axon_ntff_profile_hook(hook)
    except Exception:
        pass


_install_axon_ntff_shim()

import concourse.bass as bass
import concourse.mybir as mybir
import concourse.tile as tile
from concourse import bacc
from concourse.bass_utils import run_bass_kernel_spmd
from concourse.masks import make_identity

F32 = mybir.dt.float32
F32R = mybir.dt.float32r
BF16 = mybir.dt.bfloat16
I16 = mybir.dt.int16

L_FULL, NH, DH, D, DI, V = 12, 12, 64, 768, 3072, 10000
QLEN, MLEN, CLEN, BSZ = 512, 512, 32, 4
KLEN = CLEN + MLEN + QLEN          # 1056
EPS = 1e-5
N_CORES = 8
TP = 2
NH_L = NH // TP                     # 6 local heads
HD_L = NH_L * DH                    # 384
DI_L = DI // TP                     # 1536
V_L = V // TP                       # 5000
SCALE = 1.0 / float(np.sqrt(DH))
SENT = -1.0e38
SKW = KLEN + QLEN - 1               # 1567
JSEGS = [(0, 32)] + [(32 + 128 * k, 128) for k in range(8)]
KCH = [(0, 512), (512, 512), (1024, 32)]
MEMCOLS = CLEN + MLEN               # 544

_BUILD_CACHE = {}


def _build(L):
    nc = bacc.Bacc("TRN2", target_bir_lowering=False, debug=False,
                   num_devices=N_CORES)

    def din(name, shape, dt):
        return nc.dram_tensor(name, shape, dt, kind="ExternalInput")

    emb_d = din("emb", [V, D], F32)
    idx_d = din("idx", [128, 32], I16)
    condT_d = din("condT", [D, CLEN], BF16)
    memsT_d = din("memsT", [L, D, MLEN], BF16)
    rT_d = din("rT", [D, KLEN], BF16)
    wq_d = din("wq", [L, D, HD_L], BF16)
    wk_d = din("wk", [L, D, HD_L], BF16)
    wv_d = din("wv", [L, D, HD_L], BF16)
    rnet_d = din("rnet", [L, D, HD_L], BF16)
    ow_d = din("ow", [L, HD_L, D], BF16)
    w1_d = din("w1", [L, D, DI_L], BF16)
    b1_d = din("b1", [L, DI_L // 128, 128], F32)
    w2_d = din("w2", [L, DI_L, D], BF16)
    b2_d = din("b2", [L, D // 128, 128], F32)
    ln1g_d = din("ln1g", [L, D // 128, 128], F32)
    ln1b_d = din("ln1b", [L, D // 128, 128], F32)
    ln2g_d = din("ln2g", [L, D // 128, 128], F32)
    ln2b_d = din("ln2b", [L, D // 128, 128], F32)
    rwb_d = din("rwb", [HD_L // 128, 128], F32)
    rrb_d = din("rrb", [HD_L // 128, 128], F32)
    projw_d = din("projw", [D, V_L], BF16)
    projb_d = din("projb", [1, V_L], F32)
    out_d = nc.dram_tensor("logits", [QLEN, V_L], F32, kind="ExternalOutput")

    skew_d = nc.dram_tensor("skew", [NH_L * QLEN * SKW + 4096], BF16,
                            kind="Internal")
    H2 = QLEN // 2
    cc_in2 = [nc.dram_tensor(f"cc_in{h}", [D, H2], BF16, kind="Internal")
              for h in range(2)]
    cc_out2 = [nc.dram_tensor(f"cc_out{h}", [D, H2], BF16, kind="Internal")
               for h in range(2)]
    RG = [[0, 1], [2, 3], [4, 5], [6, 7]]

    with tile.TileContext(nc) as tc:
        import contextlib
        ctx = contextlib.ExitStack()
        with ctx:
            ctx.enter_context(nc.allow_low_precision("bf16 kernel by design"))
            P = 128
            const = ctx.enter_context(tc.tile_pool(name="const", bufs=1))
            persist = ctx.enter_context(tc.tile_pool(name="persist", bufs=1))
            lw = ctx.enter_context(tc.tile_pool(name="lw", bufs=1))
            wstream = ctx.enter_context(tc.tile_pool(name="wstream", bufs=2))
            pstream = ctx.enter_context(tc.tile_pool(name="pstream", bufs=2))
            work = ctx.enter_context(tc.tile_pool(name="work", bufs=2))
            big1 = ctx.enter_context(tc.tile_pool(name="big1", bufs=1))
            small = ctx.enter_context(tc.tile_pool(name="small", bufs=1))
            smalls = ctx.enter_context(tc.tile_pool(name="smalls", bufs=4))
            ps_big = ctx.enter_context(tc.tile_pool(name="psb", bufs=2, space="PSUM"))
            ps_sm = ctx.enter_context(tc.tile_pool(name="pss", bufs=2, space="PSUM"))

            # ---- constants ----
            ident_f = const.tile([P, P], F32)
            make_identity(nc, ident_f)
            ident_b = const.tile([P, P], BF16)
            nc.vector.tensor_copy(out=ident_b, in_=ident_f)
            ones_f = const.tile([P, 1], F32)
            nc.vector.memset(ones_f, 1.0)
            ones_b = const.tile([P, 1], BF16)
            nc.vector.tensor_copy(out=ones_b, in_=ones_f)
            ones_r = const.tile([P, 1], F32R)
            nc.vector.tensor_copy(out=ones_r, in_=ones_f)
            onesrow_f = const.tile([1, P], F32)
            nc.vector.memset(onesrow_f, 1.0)
            onesrow_r = const.tile([1, P], F32R)
            nc.vector.tensor_copy(out=onesrow_r, in_=onesrow_f)
            sent_t = const.tile([P, QLEN - 1], BF16)
            nc.vector.memset(sent_t, SENT)
            rwb_t = const.tile([P, 3], F32)
            nc.sync.dma_start(out=rwb_t, in_=rwb_d.ap().rearrange("k p -> p k"))
            rrb_t = const.tile([P, 3], F32)
            nc.sync.dma_start(out=rrb_t, in_=rrb_d.ap().rearrange("k p -> p k"))
            idxs = const.tile([P, 32], I16)
            nc.sync.dma_start(out=idxs, in_=idx_d.ap())
            eps_t = const.tile([1, 1], F32)
            nc.vector.memset(eps_t, EPS)
            eps_c = const.tile([P, 1], F32)
            nc.vector.memset(eps_c, EPS)
            projb_sb = const.tile([1, V_L], F32R)
            nc.gpsimd.dma_start(out=projb_sb, in_=projb_d.ap())

            # ---- persistent activations (bf16 unless noted) ----
            condT = persist.tile([P, 6, CLEN], BF16)
            nc.sync.dma_start(out=condT, in_=condT_d.ap().rearrange("(k p) t -> p k t", p=P))
            rT = persist.tile([P, 6, KLEN], BF16)
            nc.sync.dma_start(out=rT, in_=rT_d.ap().rearrange("(k p) t -> p k t", p=P))
            hT = persist.tile([P, 6, QLEN], BF16)
            h1 = persist.tile([P, 6, QLEN], BF16)
            qrw = persist.tile([P, 3, QLEN], BF16)
            qrr = persist.tile([P, 3, QLEN], BF16)
            probT = persist.tile([P, 9, QLEN], BF16)
            nc.vector.memset(probT, 0.0)
            av_sb = persist.tile([P, 3, QLEN], BF16)
            # double-buffered across layers (prologue of l+1 overlaps body of l)
            kT2 = [persist.tile([P, 3, KLEN], BF16, tag=f"kT{i}", name=f"kT{i}")
                   for i in range(2)]
            rkT2 = [persist.tile([P, 3, KLEN], BF16, tag=f"rkT{i}", name=f"rkT{i}")
                    for i in range(2)]
            vt2 = [persist.tile([P, 9, HD_L], BF16, tag=f"vt{i}", name=f"vt{i}")
                   for i in range(2)]

            # ---- init skew buffer pad region [KLEN, SKW) with sentinel ----
            for n in range(NH_L):
                for t in range(4):
                    dst = bass.AP(tensor=skew_d.ap().tensor,
                                  offset=n * QLEN * SKW + t * 128 * SKW + KLEN,
                                  ap=[[SKW, 128], [1, QLEN - 1]])
                    nc.gpsimd.dma_start(out=dst, in_=sent_t)

            # ---- per-layer weight tiles (single slot per tag, rotated) ----
            W = {}

            def load_early(l):
                """Weights needed by prologue(l): issue during layer l-1."""
                d = W.setdefault(l, {})
                d['rnet'] = lw.tile([P, 6, HD_L], BF16, tag="rnet", name="rnet")
                nc.sync.dma_start(out=d['rnet'], in_=rnet_d.ap()[l].rearrange("(k p) m -> p k m", p=P))
                d['wk'] = lw.tile([P, 6, HD_L], BF16, tag="wk", name="wk")
                nc.sync.dma_start(out=d['wk'], in_=wk_d.ap()[l].rearrange("(k p) m -> p k m", p=P))
                d['wv'] = lw.tile([P, 6, HD_L], BF16, tag="wv", name="wv")
                nc.sync.dma_start(out=d['wv'], in_=wv_d.ap()[l].rearrange("(k p) m -> p k m", p=P))
                d['memsT'] = lw.tile([P, 6, MLEN], BF16, tag="memsT", name="memsT")
                nc.sync.dma_start(out=d['memsT'], in_=memsT_d.ap()[l].rearrange("(k p) t -> p k t", p=P))

            def load_late(l):
                """wq/ow for layer l — their layer l-1 uses are early in the body."""
                d = W.setdefault(l, {})
                d['wq'] = lw.tile([P, 6, HD_L], BF16, tag="wq", name="wq")
                nc.sync.dma_start(out=d['wq'], in_=wq_d.ap()[l].rearrange("(k p) m -> p k m", p=P))
                d['ow'] = lw.tile([P, 3, D], BF16, tag="ow", name="ow")
                nc.sync.dma_start(out=d['ow'], in_=ow_d.ap()[l].rearrange("(k p) m -> p k m", p=P))

            def load_small(l):
                """Biases/LN params — must be emitted after layer l-1's LN2."""
                d = W.setdefault(l, {})
                d['b1'] = lw.tile([P, 12], F32, tag="b1", name="b1")
                nc.sync.dma_start(out=d['b1'], in_=b1_d.ap()[l].rearrange("k p -> p k"))
                d['b2'] = lw.tile([P, 6], F32, tag="b2", name="b2")
                nc.sync.dma_start(out=d['b2'], in_=b2_d.ap()[l].rearrange("k p -> p k"))
                d['ln1g'] = lw.tile([P, 6], F32, tag="ln1g", name="ln1g")
                nc.sync.dma_start(out=d['ln1g'], in_=ln1g_d.ap()[l].rearrange("k p -> p k"))
                d['ln1b'] = lw.tile([P, 6], F32, tag="ln1b", name="ln1b")
                nc.sync.dma_start(out=d['ln1b'], in_=ln1b_d.ap()[l].rearrange("k p -> p k"))
                d['ln2g'] = lw.tile([P, 6], F32, tag="ln2g", name="ln2g")
                nc.sync.dma_start(out=d['ln2g'], in_=ln2g_d.ap()[l].rearrange("k p -> p k"))
                d['ln2b'] = lw.tile([P, 6], F32, tag="ln2b", name="ln2b")
                nc.sync.dma_start(out=d['ln2b'], in_=ln2b_d.ap()[l].rearrange("k p -> p k"))

            def prologue_A(l):
                """h-independent: rkT (full) and kT cond+mems columns [0, 544)."""
                d = W[l]
                kT, rkT = kT2[l % 2], rkT2[l % 2]
                for m in range(3):
                    pk = ps_big.tile([P, KLEN], F32, tag="big")
                    for k in range(6):
                        st, sp = (k == 0), (k == 5)
                        lhs = d['rnet'][:, k, m * P:(m + 1) * P]
                        for (c0, w) in KCH:
                            nc.tensor.matmul(pk[:, c0:c0 + w], lhs, rT[:, k, c0:c0 + w],
                                             start=st, stop=sp)
                    nc.scalar.copy(out=rkT[:, m, :], in_=pk)
                for m in range(3):
                    pk = ps_big.tile([P, MEMCOLS], F32, tag="big")
                    for k in range(6):
                        st, sp = (k == 0), (k == 5)
                        lhs = d['wk'][:, k, m * P:(m + 1) * P]
                        nc.tensor.matmul(pk[:, 0:32], lhs, condT[:, k, :], start=st, stop=sp)
                        nc.tensor.matmul(pk[:, 32:512], lhs, d['memsT'][:, k, 0:480], start=st, stop=sp)
                        nc.tensor.matmul(pk[:, 512:544], lhs, d['memsT'][:, k, 480:512], start=st, stop=sp)
                    nc.scalar.copy(out=kT[:, m, 0:MEMCOLS], in_=pk)

            def prologue_B(l):
                """h-independent v segments (cond + mems tokens)."""
                d = W[l]
                vt = vt2[l % 2]
                for s in range(5):
                    off, w = JSEGS[s]
                    pv = ps_sm.tile([P, HD_L], F32, tag="sm")
                    if s == 0:
                        src, soff = condT, 0
                    else:
                        src, soff = d['memsT'], off - 32
                    for k in range(6):
                        nc.tensor.matmul(pv[0:w, :], src[:, k, soff:soff + w],
                                         d['wv'][:, k, :], start=(k == 0), stop=(k == 5))
                    nc.vector.tensor_copy(out=vt[0:w, s, :], in_=pv[0:w, :])

            def ln_dmajor(src_t, g_sb, b_sb, out_t, c0=0, w=QLEN):
                """LayerNorm over D for d-major [128, 6, QLEN] bf16 src,
                restricted to token columns [c0, c0+w).

                Stats are transposed into column form so divide / sqrt run
                on all 128 DVE lanes, then transposed back for the
                broadcast matmuls."""
                nch = w // P
                s1 = ps_sm.tile([1, w], F32, tag="sm", name="lns1")
                for k in range(6):
                    nc.tensor.matmul(s1, ones_b, src_t[:, k, c0:c0 + w],
                                     start=(k == 0), stop=(k == 5))
                s2 = ps_sm.tile([1, w], F32, tag="sm", name="lns2")
                for k in range(6):
                    sq = work.tile([P, w], F32R, tag="lnsq", name="lnsq")
                    nc.vector.tensor_mul(out=sq, in0=src_t[:, k, c0:c0 + w],
                                         in1=src_t[:, k, c0:c0 + w])
                    nc.tensor.matmul(s2, ones_r, sq,
                                     start=(k == 0), stop=(k == 5))
                mean = small.tile([1, w], F32, tag="mean", name="mean")
                nc.scalar.mul(out=mean, in_=s1, mul=1.0 / D)
                e2 = small.tile([1, w], F32, tag="e2", name="e2")
                nc.scalar.mul(out=e2, in_=s2, mul=1.0 / D)
                sT = ps_sm.tile([P, 2 * nch], F32, tag="sm", name="lnsT")
                for c in range(nch):
                    nc.tensor.matmul(sT[:, c:c + 1], mean[0:1, c * P:(c + 1) * P],
                                     ones_f[0:1, 0:1], start=True, stop=True)
                    nc.tensor.matmul(sT[:, nch + c:nch + c + 1],
                                     e2[0:1, c * P:(c + 1) * P],
                                     ones_f[0:1, 0:1], start=True, stop=True)
                stats = smalls.tile([P, 2 * nch], F32, tag="stats", name="stats")
                nc.vector.tensor_copy(out=stats, in_=sT)
                varT = smalls.tile([P, nch], F32, tag="varT", name="varT")
                nc.vector.tensor_mul(out=varT, in0=stats[:, 0:nch], in1=stats[:, 0:nch])
                nc.vector.tensor_sub(out=varT, in0=stats[:, nch:2 * nch], in1=varT)
                nc.scalar.activation(out=varT, in_=varT,
                                     func=mybir.ActivationFunctionType.Sqrt,
                                     bias=eps_c, scale=1.0)
                rstdT = smalls.tile([P, nch], F32, tag="rstdT", name="rstdT")
                nc.vector.reciprocal(out=rstdT, in_=varT)
                rsp = ps_sm.tile([1, w], F32, tag="sm", name="lnrsp")
                for c in range(nch):
                    nc.tensor.matmul(rsp[0:1, c * P:(c + 1) * P], rstdT[:, c:c + 1],
                                     ident_f, start=True, stop=True)
                rstd = small.tile([1, w], F32, tag="rstd", name="rstd")
                nc.vector.tensor_copy(out=rstd, in_=rsp)
                meanB = ps_sm.tile([P, w], F32, tag="sm", name="lnmB")
                nc.tensor.matmul(meanB, onesrow_f, mean, start=True, stop=True)
                rstdB = ps_sm.tile([P, w], F32, tag="sm", name="lnrB")
                nc.tensor.matmul(rstdB, onesrow_f, rstd, start=True, stop=True)
                for k in range(6):
                    tmp = work.tile([P, w], F32, tag="lnt", name="lnt")
                    nc.vector.tensor_sub(out=tmp, in0=src_t[:, k, c0:c0 + w], in1=meanB)
                    nc.vector.tensor_mul(out=tmp, in0=tmp, in1=rstdB)
                    nc.vector.tensor_scalar(out=out_t[:, k, c0:c0 + w], in0=tmp,
                                            scalar1=g_sb[:, k:k+1],
                                            scalar2=b_sb[:, k:k+1],
                                            op0=mybir.AluOpType.mult,
                                            op1=mybir.AluOpType.add)

            # ================== preamble: layer 0 prologue ==================
            load_early(0)
            load_late(0)
            load_small(0)
            prologue_A(0)
            prologue_B(0)

            # ---- embedding: gather, transpose to d-major, scale ----
            gath = big1.tile([P, 4, D], F32, tag="big12")
            nc.gpsimd.dma_gather(out_ap=gath, in_ap=emb_d.ap(), idxs_ap=idxs,
                                 num_idxs=QLEN, num_idxs_reg=QLEN, elem_size=D)
            for it in range(4):
                for dt_ in range(6):
                    pt = ps_sm.tile([P, P], F32, tag="sm")
                    nc.tensor.transpose(pt, gath[:, it, dt_ * P:(dt_ + 1) * P], ident_f)
                    nc.scalar.mul(out=hT[:, dt_, it * P:(it + 1) * P], in_=pt,
                                  mul=float(np.sqrt(D)))

            # valid BD cols for i-tile t: [384-128t, 1056); AC: [0, 673+128t)
            BDCH = {0: [(384, 128), (512, 512), (1024, 32)],
                    1: [(256, 256), (512, 512), (1024, 32)],
                    2: [(128, 384), (512, 512), (1024, 32)],
                    3: [(0, 512), (512, 512), (1024, 32)]}
            ACCH = {0: [(0, 512), (512, 161)],
                    1: [(0, 512), (512, 289)],
                    2: [(0, 512), (512, 417)],
                    3: [(0, 512), (512, 512), (1024, 32)]}
            # transpose segments actually reachable for i-tile t (rest stay 0)
            NSEG_T = {0: 7, 1: 8, 2: 9, 3: 9}

            # ============================ layers ============================
            for l in range(L):
                d = W[l]
                kT, rkT, v_tok = kT2[l % 2], rkT2[l % 2], vt2[l % 2]

                # ---- kT h-derived columns [544, 1056) ----
                for m in range(3):
                    pk = ps_sm.tile([P, QLEN], F32, tag="sm")
                    for k in range(6):
                        st, sp = (k == 0), (k == 5)
                        lhs = d['wk'][:, k, m * P:(m + 1) * P]
                        nc.tensor.matmul(pk[:, 0:480], lhs, hT[:, k, 0:480], start=st, stop=sp)
                        nc.tensor.matmul(pk[:, 480:512], lhs, hT[:, k, 480:512], start=st, stop=sp)
                    nc.scalar.copy(out=kT[:, m, MEMCOLS:KLEN], in_=pk)

                # ---- q + rel biases ----
                for m in range(3):
                    pq = ps_sm.tile([P, QLEN], F32, tag="sm")
                    for k in range(6):
                        nc.tensor.matmul(pq, d['wq'][:, k, m * P:(m + 1) * P],
                                         hT[:, k, :], start=(k == 0), stop=(k == 5))
                    nc.vector.tensor_scalar_add(out=qrw[:, m, :], in0=pq,
                                                scalar1=rwb_t[:, m:m+1])
                    nc.vector.tensor_scalar_add(out=qrr[:, m, :], in0=pq,
                                                scalar1=rrb_t[:, m:m+1])

                # ---- v h-derived segments 5..8 ----
                for s in range(5, 9):
                    off, w = JSEGS[s]
                    pv = ps_sm.tile([P, HD_L], F32, tag="sm")
                    soff = off - MEMCOLS
                    for k in range(6):
                        nc.tensor.matmul(pv[0:w, :], hT[:, k, soff:soff + w],
                                         d['wv'][:, k, :], start=(k == 0), stop=(k == 5))
                    nc.vector.tensor_copy(out=v_tok[0:w, s, :], in_=pv[0:w, :])

                # ---- prefetch next layer's prologue weights ----
                if l + 1 < L:
                    load_early(l + 1)

                # ---- attention ----
                def bd_unit(n, t):
                    hp0 = 64 * (n % 2)
                    hk = n // 2
                    c0min = BDCH[t][0][0]
                    pb = ps_big.tile([P, KLEN], F32, tag="big")
                    lhs = qrr[hp0:hp0 + 64, hk, t * P:(t + 1) * P]
                    for (c0, w) in BDCH[t]:
                        nc.tensor.matmul(pb[:, c0:c0 + w], lhs,
                                         rkT[hp0:hp0 + 64, hk, c0:c0 + w],
                                         start=True, stop=True)
                    bd_i = work.tile([P, KLEN], BF16, tag="bdi", bufs=3)
                    mid = max(c0min, 512)
                    nc.vector.tensor_copy(out=bd_i[:, c0min:mid], in_=pb[:, c0min:mid])
                    nc.scalar.copy(out=bd_i[:, mid:], in_=pb[:, mid:])
                    dst = bass.AP(tensor=skew_d.ap().tensor,
                                  offset=n * QLEN * SKW + t * 128 * SKW + c0min,
                                  ap=[[SKW, 128], [1, KLEN - c0min]])
                    nc.gpsimd.dma_start(out=dst, in_=bd_i[:, c0min:])

                def score_unit(n, t):
                    hp0 = 64 * (n % 2)
                    hk = n // 2
                    bd_s = work.tile([P, KLEN], BF16, tag="bds", bufs=3)
                    src = bass.AP(tensor=skew_d.ap().tensor,
                                  offset=n * QLEN * SKW + t * 128 * (SKW - 1) + QLEN - 1,
                                  ap=[[SKW - 1, 128], [1, KLEN]])
                    nc.scalar.dma_start(out=bd_s, in_=src)
                    pa = ps_big.tile([P, KLEN], F32, tag="big")
                    lhs = qrw[hp0:hp0 + 64, hk, t * P:(t + 1) * P]
                    for (c0, w) in KCH:
                        nc.tensor.matmul(pa[:, c0:c0 + w], ident_b,
                                         bd_s[:, c0:c0 + w], start=True, stop=False)
                    for (c0, w) in ACCH[t]:
                        nc.tensor.matmul(pa[:, c0:c0 + w], lhs,
                                         kT[hp0:hp0 + 64, hk, c0:c0 + w],
                                         start=False, stop=True)
                    prob = work.tile([P, KLEN], BF16, tag="prob", bufs=3)
                    dnm = smalls.tile([P, 1], F32, tag="dnm")
                    nc.scalar.activation(out=prob, in_=pa,
                                         func=mybir.ActivationFunctionType.Exp,
                                         bias=0.0, scale=SCALE, accum_out=dnm)
                    rd = smalls.tile([P, 1], F32, tag="rd")
                    nc.vector.reciprocal(out=rd, in_=dnm)
                    nc.vector.tensor_scalar_mul(out=prob, in0=prob, scalar1=rd)
                    for s in range(NSEG_T[t]):
                        off, w = JSEGS[s]
                        ptr = ps_sm.tile([P, P], BF16, tag="sm")
                        nc.tensor.transpose(ptr[0:w, :], prob[:, off:off + w], ident_b)
                        nc.any.tensor_copy(out=probT[0:w, s, t * P:(t + 1) * P],
                                           in_=ptr[0:w, :])

                def av_unit(n):
                    hp0 = 64 * (n % 2)
                    hk = n // 2
                    pav = ps_sm.tile([64, QLEN], F32, tag="sm")
                    for s in range(9):
                        off, w = JSEGS[s]
                        nc.tensor.matmul(pav, v_tok[0:w, s, 64 * n:64 * n + 64],
                                         probT[0:w, s, :],
                                         start=(s == 0), stop=(s == 8))
                    nc.vector.tensor_copy(out=av_sb[hp0:hp0 + 64, hk, :], in_=pav)

                # phase A with a 3-unit lag before phase B per head
                for n in range(NH_L):
                    for t in range(4):
                        bd_unit(n, t)
                    if n >= 1:
                        for t in range(4):
                            score_unit(n - 1, t)
                        av_unit(n - 1)
                for t in range(4):
                    score_unit(NH_L - 1, t)
                av_unit(NH_L - 1)

                # ---- o-proj token halves -> pipelined AllReduces ----
                for half in range(2):
                    c0 = half * H2
                    for m in range(6):
                        po = ps_sm.tile([P, H2], F32, tag="sm", name="po")
                        for k in range(3):
                            nc.tensor.matmul(po, d['ow'][:, k, m * P:(m + 1) * P],
                                             av_sb[:, k, c0:c0 + H2],
                                             start=(k == 0), stop=(k == 2))
                        ob = work.tile([P, H2], BF16, tag="ob", name="ob")
                        nc.vector.tensor_copy(out=ob, in_=po)
                        nc.gpsimd.dma_start(
                            out=cc_in2[half].ap().rearrange("(k p) t -> p k t", p=P)[:, m, :],
                            in_=ob)
                    nc.gpsimd.collective_compute(
                        "AllReduce", mybir.AluOpType.add, replica_groups=RG,
                        ins=[cc_in2[half].ap()], outs=[cc_out2[half].ap()])

                # ---- fill AR1 stall: next layer's h-independent compute ----
                if l + 1 < L:
                    load_late(l + 1)
                    prologue_A(l + 1)

                # ---- residual+LN1+FFN1 per half as its AllReduce lands ----
                ffn1 = big1.tile([P, 12, QLEN], BF16, tag="big12")
                for half in range(2):
                    c0 = half * H2
                    nc.gpsimd.dma_start(
                        out=hT[:, :, c0:c0 + H2],
                        in_=cc_out2[half].ap().rearrange("(k p) t -> p k t", p=P),
                        accum_op=mybir.AluOpType.add)
                    ln_dmajor(hT, d['ln1g'], d['ln1b'], h1, c0, H2)
                    for km in range(12):
                        pf = ps_sm.tile([P, H2], F32, tag="sm", name="pf")
                        wsl = wstream.tile([P, 6, P], BF16, tag="w1s", name="w1s", bufs=3)
                        src = bass.AP(tensor=w1_d.ap().tensor,
                                      offset=l * D * DI_L + km * P,
                                      ap=[[DI_L, P], [P * DI_L, 6], [1, P]])
                        nc.sync.dma_start(out=wsl, in_=src)
                        for k in range(6):
                            nc.tensor.matmul(pf, wsl[:, k, :], h1[:, k, c0:c0 + H2],
                                             start=(k == 0), stop=(k == 5))
                        nc.scalar.activation(out=ffn1[:, km, c0:c0 + H2], in_=pf,
                                             func=mybir.ActivationFunctionType.Relu,
                                             bias=d['b1'][:, km:km+1], scale=1.0)

                # ---- FFN2 per half -> pipelined AllReduces ----
                for half in range(2):
                    c0 = half * H2
                    for m in range(6):
                        pf = ps_sm.tile([P, H2], F32, tag="sm", name="pf2")
                        wsl = wstream.tile([P, 12, P], BF16, tag="w2s", name="w2s")
                        src = bass.AP(tensor=w2_d.ap().tensor,
                                      offset=l * DI_L * D + m * P,
                                      ap=[[D, P], [P * D, 12], [1, P]])
                        nc.sync.dma_start(out=wsl, in_=src)
                        for k in range(12):
                            nc.tensor.matmul(pf, wsl[:, k, :], ffn1[:, k, c0:c0 + H2],
                                             start=(k == 0), stop=(k == 11))
                        fb = work.tile([P, H2], BF16, tag="ob", name="fb")
                        nc.vector.tensor_scalar_add(out=fb, in0=pf,
                                                    scalar1=d['b2'][:, m:m+1])
                        nc.gpsimd.dma_start(
                            out=cc_in2[half].ap().rearrange("(k p) t -> p k t", p=P)[:, m, :],
                            in_=fb)
                    nc.gpsimd.collective_compute(
                        "AllReduce", mybir.AluOpType.add, replica_groups=RG,
                        ins=[cc_in2[half].ap()], outs=[cc_out2[half].ap()])

                # ---- fill AR2 stall: next layer's v cond+mems segments ----
                if l + 1 < L:
                    prologue_B(l + 1)

                for half in range(2):
                    c0 = half * H2
                    nc.gpsimd.dma_start(
                        out=h1[:, :, c0:c0 + H2],
                        in_=cc_out2[half].ap().rearrange("(k p) t -> p k t", p=P),
                        accum_op=mybir.AluOpType.add)
                    ln_dmajor(h1, d['ln2g'], d['ln2b'], hT, c0, H2)
                if l + 1 < L:
                    load_small(l + 1)

            # ---- final projection (token-major out) ----
            NCH = 500
            for tt in range(4):
                for c in range(V_L // NCH):
                    pp = ps_sm.tile([P, NCH], F32, tag="sm")
                    wsl = pstream.tile([P, 6, NCH], BF16, tag="pws")
                    src = bass.AP(tensor=projw_d.ap().tensor,
                                  offset=c * NCH,
                                  ap=[[V_L, P], [P * V_L, 6], [1, NCH]])
                    nc.sync.dma_start(out=wsl, in_=src)
                    for k in range(6):
                        nc.tensor.matmul(pp, hT[:, k, tt * P:(tt + 1) * P],
                                         wsl[:, k, :], start=(k == 0), stop=False)
                    pbs = small.tile([1, NCH], F32R, tag="pbs")
                    nc.vector.tensor_copy(out=pbs, in_=projb_sb[:, c * NCH:(c + 1) * NCH])
                    nc.tensor.matmul(pp, onesrow_r, pbs, start=False, stop=True)
                    osb = work.tile([P, NCH], F32, tag="osb")
                    nc.vector.tensor_copy(out=osb, in_=pp)
                    nc.sync.dma_start(out=out_d.ap()[tt * P:(tt + 1) * P,
                                                     c * NCH:(c + 1) * NCH],
                                      in_=osb)

    nc.compile()
    return nc


def _pos_emb_T(klen):
    pos = np.arange(klen - 1, -1, -1, dtype=np.float32)
    inv = 1.0 / (10000.0 ** (np.arange(0, D, 2, dtype=np.float32) / D))
    s = pos[:, None] * inv[None, :]
    r = np.concatenate([np.sin(s), np.cos(s)], axis=-1)
    return np.ascontiguousarray(r.T)  # [D, klen]


def kernel(x, condition, mems, emb, qkv_w, r_net_w, o_w, ln1_g, ln1_b,
           w1, b1, w2, b2, ln2_g, ln2_b, r_w_bias, r_r_bias, proj_w, proj_b):
    import ml_dtypes
    BF = ml_dtypes.bfloat16

    L = int(os.environ.get("KERNEL_LAYERS", str(L_FULL)))
    if L not in _BUILD_CACHE:
        _BUILD_CACHE[L] = _build(L)
    nc = _BUILD_CACHE[L]

    f32 = lambda a: np.asarray(a, dtype=np.float32)
    bf = lambda a: np.ascontiguousarray(np.asarray(a, dtype=np.float32).astype(BF))
    x = np.asarray(x)
    condition = f32(condition); mems = f32(mems); emb = f32(emb)
    qkv_w = f32(qkv_w); r_net_w = f32(r_net_w); o_w = f32(o_w)
    ln1_g = f32(ln1_g); ln1_b = f32(ln1_b); w1 = f32(w1); b1 = f32(b1)
    w2 = f32(w2); b2 = f32(b2); ln2_g = f32(ln2_g); ln2_b = f32(ln2_b)
    r_w_bias = f32(r_w_bias); r_r_bias = f32(r_r_bias)
    proj_w = f32(proj_w); proj_b = f32(proj_b)

    rT = _pos_emb_T(KLEN)

    in_maps = []
    for c in range(N_CORES):
        b, half = c // TP, c % TP
        toks = np.asarray(x[:, b], dtype=np.int64)
        idxw = np.zeros((128, 32), np.int16)
        ar = toks.reshape(32, 16).astype(np.int16)  # token i = col*16 + row
        for k in range(8):
            idxw[16 * k:16 * (k + 1), :] = ar.T
        hs = slice(half * HD_L, (half + 1) * HD_L)
        fs = slice(half * DI_L, (half + 1) * DI_L)
        vs = slice(half * V_L, (half + 1) * V_L)
        b2v = b2 if half == 0 else np.zeros_like(b2)
        m = {
            "emb": np.ascontiguousarray(emb),
            "idx": idxw,
            "condT": bf(condition[:, b, :].T),
            "memsT": bf(mems[:L, :, b, :].transpose(0, 2, 1)),
            "rT": bf(rT),
            "wq": bf(qkv_w[:L, :, hs]),
            "wk": bf(qkv_w[:L, :, 768 + half * HD_L:768 + (half + 1) * HD_L]),
            "wv": bf(qkv_w[:L, :, 1536 + half * HD_L:1536 + (half + 1) * HD_L]),
            "rnet": bf(r_net_w[:L, :, hs]),
            "ow": bf(o_w[:L, hs, :]),
            "w1": bf(w1[:L, :, fs]),
            "b1": np.ascontiguousarray(b1[:L, fs]).reshape(L, 12, 128),
            "w2": bf(w2[:L, fs, :]),
            "b2": np.ascontiguousarray(b2v[:L]).reshape(L, 6, 128),
            "ln1g": np.ascontiguousarray(ln1_g[:L]).reshape(L, 6, 128),
            "ln1b": np.ascontiguousarray(ln1_b[:L]).reshape(L, 6, 128),
            "ln2g": np.ascontiguousarray(ln2_g[:L]).reshape(L, 6, 128),
            "ln2b": np.ascontiguousarray(ln2_b[:L]).reshape(L, 6, 128),
            "rwb": np.ascontiguousarray(r_w_bias.reshape(NH * DH)[half * HD_L:(half + 1) * HD_L]).reshape(3, 128),
            "rrb": np.ascontiguousarray(r_r_bias.reshape(NH * DH)[half * HD_L:(half + 1) * HD_L]).reshape(3, 128),
            "projw": bf(proj_w[:, vs]),
            "projb": np.ascontiguousarray(proj_b[vs]).reshape(1, V_L),
        }
        in_maps.append(m)

    trace = bool(int(os.environ.get("KERNEL_TRACE", "0")))
    res = run_bass_kernel_spmd(nc, in_maps, core_ids=list(range(N_CORES)),
                               trace=trace)
    kernel.last_result = res

    out = np.zeros((QLEN, BSZ, V), np.float32)
    for c in range(N_CORES):
        b, half = c // TP, c % TP
        out[:, b, half * V_L:(half + 1) * V_L] = res.results[c]["logits"]
    return out



# revision 21
# speedup vs baseline: 1.0027x; 1.0027x over previous
"""MemTransformerLM (Transformer-XL) forward on 8 Trainium2 NeuronCores.

v4: data-parallel over batch (4) x tensor-parallel over heads (2) for
attention; sequence-parallel FFN within each pair (ReduceScatter after
o-proj -> full-d_inner FFN on own 256 tokens -> AllGather), replacing the
two AllReduces per layer.  Residual h/2 rides inside the ReduceScatter so
all addressing stays SPMD-uniform.

Attention is computed transposed (scoreT[j,i]) so attention-weighted
values need no PE transposes: the rel-shift skew buffer is read back
through the DMA xbar transpose, softmax denominators come from a ones
column appended to V (M=65 matmuls), normalization happens at fp32 in
PSUM.  Score = AC+BD add runs on DVE instead of identity matmuls.

Final projection is token-parallel (own 256 tokens x full vocab): no
final AllGather.

Numerics: bf16 storage / fp32 PSUM accumulation; LayerNorm stats fp32.
Self-contained: hardcodes all shapes; takes full inputs, returns full output.
"""
import os
import sys
import types

sys.path.insert(0, '/opt/trn_rl_repo')

import numpy as np


def _install_axon_ntff_shim():
    try:
        from antenv import axon_hooks  # noqa: F401
        return
    except ImportError:
        pass
    try:
        import antenv
        mod = types.ModuleType("antenv.axon_hooks")
        mod._hook = None

        def _set(h):
            mod._hook = h

        def _get():
            return mod._hook

        mod.set_axon_ntff_profile_hook = _set
        mod.get_axon_ntff_profile_hook = _get
        sys.modules["antenv.axon_hooks"] = mod
        antenv.axon_hooks = mod
        from trn_agent_boot.trn_boot import _ntff_profile_via_ctypes
        hook = _ntff_profile_via_ctypes('/opt/axon/libaxon_pjrt.so')
        if hook is not None:
            mod.set_axon_ntff_profile_hook(hook)
    except Exception:
        pass


_install_axon_ntff_shim()

import concourse.bass as bass
import concourse.mybir as mybir
import concourse.tile as tile
from concourse import bacc
from concourse.bass_utils import run_bass_kernel_spmd
from concourse.masks import make_identity

F32 = mybir.dt.float32
F32R = mybir.dt.float32r
BF16 = mybir.dt.bfloat16
I16 = mybir.dt.int16

L_FULL, NH, DH, D, DI, V = 12, 12, 64, 768, 3072, 10000
QLEN, MLEN, CLEN, BSZ = 512, 512, 32, 4
KLEN = CLEN + MLEN + QLEN          # 1056
EPS = 1e-5
N_CORES = 8
TP = 2
NH_L = NH // TP                     # 6 local heads
HD_L = NH_L * DH                    # 384
H2 = QLEN // 2                      # 256 own tokens
SCALE = 1.0 / float(np.sqrt(DH))
SENT = -1.0e38
SKW = KLEN + QLEN - 1               # 1567
MEMCOLS = CLEN + MLEN               # 544
KCH = [(0, 512), (512, 512), (1024, 32)]
# valid BD cols (raw d index) for i-tile t: [384-128t, 1056)
BDCH = {0: [(384, 128), (512, 512), (1024, 32)],
        1: [(256, 256), (512, 512), (1024, 32)],
        2: [(128, 384), (512, 512), (1024, 32)],
        3: [(0, 512), (512, 512), (1024, 32)]}
# j-tiles for transposed score: 8 x 128 + corner (1024, 32)
ISTART = [0, 0, 0, 0, 0, 96, 224, 352]          # first valid i col per jt
# v_tok segments: token range [128s, 128s+ws), pieces (row0, nrows, src, soff)
VSEG_PRO = {0: [(0, 32, 'cond', 0), (32, 32, 'mems', 0), (64, 64, 'mems', 32)],
            1: [(0, 128, 'mems', 96)],
            2: [(0, 128, 'mems', 224)],
            3: [(0, 128, 'mems', 352)],
            4: [(0, 32, 'mems', 480)]}
VSEG_BODY = {4: [(32, 32, 'h', 0), (64, 64, 'h', 32)],
             5: [(0, 128, 'h', 96)],
             6: [(0, 128, 'h', 224)],
             7: [(0, 128, 'h', 352)],
             8: [(0, 32, 'h', 480)]}
VW = 65                              # 64 dh + ones column per head

_BUILD_CACHE = {}


def _build(L):
    nc = bacc.Bacc("TRN2", target_bir_lowering=False, debug=False,
                   num_devices=N_CORES)

    def din(name, shape, dt):
        return nc.dram_tensor(name, shape, dt, kind="ExternalInput")

    emb_d = din("emb", [V, D], F32)
    idx_d = din("idx", [128, 32], I16)
    condT_d = din("condT", [D, CLEN], BF16)
    memsT_d = din("memsT", [L, D, MLEN], BF16)
    rT_d = din("rT", [D, KLEN], BF16)
    wq_d = din("wq", [L, D, HD_L], BF16)
    wk_d = din("wk", [L, D, HD_L], BF16)
    wv_d = din("wv", [L, D, HD_L], BF16)
    rnet_d = din("rnet", [L, D, HD_L], BF16)
    ow_d = din("ow", [L, HD_L, D], BF16)
    w1_d = din("w1", [L, D, DI], BF16)
    b1_d = din("b1", [L, DI // 128, 128], F32)
    w2_d = din("w2", [L, DI, D], BF16)
    b2_d = din("b2", [L, D // 128, 128], F32)
    ln1g_d = din("ln1g", [L, D // 128, 128], F32)
    ln1b_d = din("ln1b", [L, D // 128, 128], F32)
    ln2g_d = din("ln2g", [L, D // 128, 128], F32)
    ln2b_d = din("ln2b", [L, D // 128, 128], F32)
    rwb_d = din("rwb", [HD_L // 128, 128], F32)
    rrb_d = din("rrb", [HD_L // 128, 128], F32)
    projw_d = din("projw", [D, V], BF16)
    projb_d = din("projb", [1, V], F32)
    out_d = nc.dram_tensor("logits", [H2, V], F32, kind="ExternalOutput")

    skew_d = nc.dram_tensor("skew", [NH_L * QLEN * SKW + 4096], BF16,
                            kind="Internal")
    cc_rs_in = nc.dram_tensor("cc_rs_in", [2, D, H2], BF16, kind="Internal")
    cc_rs_out = nc.dram_tensor("cc_rs_out", [D, H2], BF16, kind="Internal")
    cc_ag_in = nc.dram_tensor("cc_ag_in", [D, H2], BF16, kind="Internal")
    cc_ag_out = nc.dram_tensor("cc_ag_out", [2, D, H2], BF16, kind="Internal")
    RG = [[0, 1], [2, 3], [4, 5], [6, 7]]

    with tile.TileContext(nc) as tc:
        import contextlib
        ctx = contextlib.ExitStack()
        with ctx:
            ctx.enter_context(nc.allow_low_precision("bf16 kernel by design"))
            P = 128
            const = ctx.enter_context(tc.tile_pool(name="const", bufs=1))
            persist = ctx.enter_context(tc.tile_pool(name="persist", bufs=1))
            lw = ctx.enter_context(tc.tile_pool(name="lw", bufs=1))
            wstream = ctx.enter_context(tc.tile_pool(name="wstream", bufs=2))
            pstream = ctx.enter_context(tc.tile_pool(name="pstream", bufs=2))
            work = ctx.enter_context(tc.tile_pool(name="work", bufs=2))
            big1 = ctx.enter_context(tc.tile_pool(name="big1", bufs=1))
            small = ctx.enter_context(tc.tile_pool(name="small", bufs=1))
            smalls = ctx.enter_context(tc.tile_pool(name="smalls", bufs=4))
            ps_pa = ctx.enter_context(tc.tile_pool(name="pspa", bufs=2, space="PSUM"))
            ps_av = ctx.enter_context(tc.tile_pool(name="psav", bufs=1, space="PSUM"))
            ps_sm = ctx.enter_context(tc.tile_pool(name="pss", bufs=2, space="PSUM"))

            # ---- constants ----
            ident_f = const.tile([P, P], F32)
            make_identity(nc, ident_f)
            ident_b = const.tile([P, P], BF16)
            nc.vector.tensor_copy(out=ident_b, in_=ident_f)
            ones_f = const.tile([P, 1], F32)
            nc.vector.memset(ones_f, 1.0)
            ones_b = const.tile([P, 1], BF16)
            nc.vector.tensor_copy(out=ones_b, in_=ones_f)
            ones_r = const.tile([P, 1], F32R)
            nc.vector.tensor_copy(out=ones_r, in_=ones_f)
            onesrow_f = const.tile([1, P], F32)
            nc.vector.memset(onesrow_f, 1.0)
            onesrow_r = const.tile([1, P], F32R)
            nc.vector.tensor_copy(out=onesrow_r, in_=onesrow_f)
            sent_t = const.tile([P, QLEN - 1], BF16)
            nc.vector.memset(sent_t, SENT)
            rwb_t = const.tile([P, 3], F32)
            nc.sync.dma_start(out=rwb_t, in_=rwb_d.ap().rearrange("k p -> p k"))
            rrb_t = const.tile([P, 3], F32)
            nc.sync.dma_start(out=rrb_t, in_=rrb_d.ap().rearrange("k p -> p k"))
            idxs = const.tile([P, 32], I16)
            nc.sync.dma_start(out=idxs, in_=idx_d.ap())
            eps_c = const.tile([P, 1], F32)
            nc.vector.memset(eps_c, EPS)

            # ---- persistent activations (bf16 unless noted) ----
            condT = persist.tile([P, 6, CLEN], BF16)
            nc.sync.dma_start(out=condT, in_=condT_d.ap().rearrange("(k p) t -> p k t", p=P))
            rT = persist.tile([P, 6, KLEN], BF16)
            nc.sync.dma_start(out=rT, in_=rT_d.ap().rearrange("(k p) t -> p k t", p=P))
            hT = persist.tile([P, 6, QLEN], BF16)
            own_pre = persist.tile([P, 6, H2], BF16)   # RS landing: h + o (own)
            h1 = persist.tile([P, 6, H2], BF16)        # post-LN1 own
            h2pre = persist.tile([P, 6, H2], BF16)     # h1 + ffn
            h_new = persist.tile([P, 6, H2], BF16)     # post-LN2 own
            qrw = persist.tile([P, 3, QLEN], BF16)
            qrr = persist.tile([P, 3, QLEN], BF16)
            av_sb = persist.tile([P, 3, QLEN], BF16)
            kT2 = [persist.tile([P, 3, KLEN], BF16, tag=f"kT{i}", name=f"kT{i}")
                   for i in range(2)]
            rkT2 = [persist.tile([P, 3, KLEN], BF16, tag=f"rkT{i}", name=f"rkT{i}")
                    for i in range(2)]
            vt2 = [persist.tile([P, 9, NH_L * VW], BF16, tag=f"vt{i}", name=f"vt{i}")
                   for i in range(2)]
            # ones columns for the softmax denominator trick: preset whole
            # tile to 1.0; value copies only overwrite the 64 dh columns.
            nc.vector.memset(vt2[0], 1.0)
            nc.vector.memset(vt2[1], 1.0)

            # ---- init skew buffer pad region [KLEN, SKW) with sentinel ----
            for n in range(NH_L):
                for t in range(4):
                    dst = bass.AP(tensor=skew_d.ap().tensor,
                                  offset=n * QLEN * SKW + t * 128 * SKW + KLEN,
                                  ap=[[SKW, 128], [1, QLEN - 1]])
                    nc.gpsimd.dma_start(out=dst, in_=sent_t)

            # ---- per-layer weight tiles (single slot per tag, rotated) ----
            W = {}

            def load_early(l):
                d = W.setdefault(l, {})
                d['rnet'] = lw.tile([P, 6, HD_L], BF16, tag="rnet", name="rnet")
                nc.sync.dma_start(out=d['rnet'], in_=rnet_d.ap()[l].rearrange("(k p) m -> p k m", p=P))
                d['wk'] = lw.tile([P, 6, HD_L], BF16, tag="wk", name="wk")
                nc.sync.dma_start(out=d['wk'], in_=wk_d.ap()[l].rearrange("(k p) m -> p k m", p=P))
                d['wv'] = lw.tile([P, 6, HD_L], BF16, tag="wv", name="wv")
                nc.sync.dma_start(out=d['wv'], in_=wv_d.ap()[l].rearrange("(k p) m -> p k m", p=P))
                d['memsT'] = lw.tile([P, 6, MLEN], BF16, tag="memsT", name="memsT")
                nc.sync.dma_start(out=d['memsT'], in_=memsT_d.ap()[l].rearrange("(k p) t -> p k t", p=P))

            def load_late(l):
                d = W.setdefault(l, {})
                d['wq'] = lw.tile([P, 6, HD_L], BF16, tag="wq", name="wq")
                nc.sync.dma_start(out=d['wq'], in_=wq_d.ap()[l].rearrange("(k p) m -> p k m", p=P))
                d['ow'] = lw.tile([P, 3, D], BF16, tag="ow", name="ow")
                nc.sync.dma_start(out=d['ow'], in_=ow_d.ap()[l].rearrange("(k p) m -> p k m", p=P))

            def load_small(l):
                d = W.setdefault(l, {})
                d['b1'] = lw.tile([P, 24], F32, tag="b1", name="b1")
                nc.sync.dma_start(out=d['b1'], in_=b1_d.ap()[l].rearrange("k p -> p k"))
                d['b2'] = lw.tile([P, 6], F32, tag="b2", name="b2")
                nc.sync.dma_start(out=d['b2'], in_=b2_d.ap()[l].rearrange("k p -> p k"))
                d['ln1g'] = lw.tile([P, 6], F32, tag="ln1g", name="ln1g")
                nc.sync.dma_start(out=d['ln1g'], in_=ln1g_d.ap()[l].rearrange("k p -> p k"))
                d['ln1b'] = lw.tile([P, 6], F32, tag="ln1b", name="ln1b")
                nc.sync.dma_start(out=d['ln1b'], in_=ln1b_d.ap()[l].rearrange("k p -> p k"))
                d['ln2g'] = lw.tile([P, 6], F32, tag="ln2g", name="ln2g")
                nc.sync.dma_start(out=d['ln2g'], in_=ln2g_d.ap()[l].rearrange("k p -> p k"))
                d['ln2b'] = lw.tile([P, 6], F32, tag="ln2b", name="ln2b")
                nc.sync.dma_start(out=d['ln2b'], in_=ln2b_d.ap()[l].rearrange("k p -> p k"))

            def v_seg(l, s, pieces):
                """Compute v_tok rows for segment s from the given pieces."""
                d = W[l]
                vt = vt2[l % 2]
                for (r0, nr, srcname, soff) in pieces:
                    pv = ps_sm.tile([P, HD_L], F32, tag="sm")
                    srcs = {'cond': condT, 'mems': d['memsT'], 'h': hT}[srcname]
                    for k in range(6):
                        nc.tensor.matmul(pv[0:nr, :],
                                         srcs[:, k, soff:soff + nr],
                                         d['wv'][:, k, :],
                                         start=(k == 0), stop=(k == 5))
                    nc.vector.tensor_copy(
                        out=vt[r0:r0 + nr, s, :].rearrange("p (h c) -> p h c", h=NH_L)[:, :, 0:DH],
                        in_=pv[0:nr, :].rearrange("p (h c) -> p h c", h=NH_L))

            def prologue_kmems(l):
                """kT cond+mems columns [0, 544)."""
                d = W[l]
                kT = kT2[l % 2]
                for m in range(3):
                    pk = ps_sm.tile([P, 512], F32, tag="sm")
                    for k in range(6):
                        st, sp = (k == 0), (k == 5)
                        lhs = d['wk'][:, k, m * P:(m + 1) * P]
                        nc.tensor.matmul(pk[:, 0:32], lhs, condT[:, k, :], start=st, stop=sp)
                        nc.tensor.matmul(pk[:, 32:512], lhs, d['memsT'][:, k, 0:480], start=st, stop=sp)
                    nc.scalar.copy(out=kT[:, m, 0:512], in_=pk)
                    pk2 = ps_sm.tile([P, 32], F32, tag="sm2", bufs=1)
                    for k in range(6):
                        nc.tensor.matmul(pk2, d['wk'][:, k, m * P:(m + 1) * P],
                                         d['memsT'][:, k, 480:512],
                                         start=(k == 0), stop=(k == 5))
                    nc.scalar.copy(out=kT[:, m, 512:MEMCOLS], in_=pk2)

            def prologue_rkT(l):
                d = W[l]
                rkT = rkT2[l % 2]
                for m in range(3):
                    for (c0, w) in KCH:
                        pk = ps_sm.tile([P, 512], F32, tag="sm")
                        for k in range(6):
                            nc.tensor.matmul(pk[:, 0:w],
                                             d['rnet'][:, k, m * P:(m + 1) * P],
                                             rT[:, k, c0:c0 + w],
                                             start=(k == 0), stop=(k == 5))
                        nc.scalar.copy(out=rkT[:, m, c0:c0 + w], in_=pk[:, 0:w])

            def ln_dmajor(src_t, g_sb, b_sb, out_t, w=H2):
                """LayerNorm over D for d-major [128, 6, w] bf16 src."""
                nch = w // P
                s1 = ps_sm.tile([1, w], F32, tag="sm", name="lns1")
                for k in range(6):
                    nc.tensor.matmul(s1, ones_b, src_t[:, k, 0:w],
                                     start=(k == 0), stop=(k == 5))
                s2 = ps_sm.tile([1, w], F32, tag="sm", name="lns2")
                for k in range(6):
                    sq = work.tile([P, w], F32R, tag="lnsq", name="lnsq")
                    nc.vector.tensor_mul(out=sq, in0=src_t[:, k, 0:w],
                                         in1=src_t[:, k, 0:w])
                    nc.tensor.matmul(s2, ones_r, sq,
                                     start=(k == 0), stop=(k == 5))
                mean = small.tile([1, w], F32, tag="mean", name="mean")
                nc.scalar.mul(out=mean, in_=s1, mul=1.0 / D)
                e2 = small.tile([1, w], F32, tag="e2", name="e2")
                nc.scalar.mul(out=e2, in_=s2, mul=1.0 / D)
                sT = ps_sm.tile([P, 2 * nch], F32, tag="sm", name="lnsT")
                for c in range(nch):
                    nc.tensor.matmul(sT[:, c:c + 1], mean[0:1, c * P:(c + 1) * P],
                                     ones_f[0:1, 0:1], start=True, stop=True)
                    nc.tensor.matmul(sT[:, nch + c:nch + c + 1],
                                     e2[0:1, c * P:(c + 1) * P],
                                     ones_f[0:1, 0:1], start=True, stop=True)
                stats = smalls.tile([P, 2 * nch], F32, tag="stats", name="stats")
                nc.vector.tensor_copy(out=stats, in_=sT)
                varT = smalls.tile([P, nch], F32, tag="varT", name="varT")
                nc.vector.tensor_mul(out=varT, in0=stats[:, 0:nch], in1=stats[:, 0:nch])
                nc.vector.tensor_sub(out=varT, in0=stats[:, nch:2 * nch], in1=varT)
                nc.scalar.activation(out=varT, in_=varT,
                                     func=mybir.ActivationFunctionType.Sqrt,
                                     bias=eps_c, scale=1.0)
                rstdT = smalls.tile([P, nch], F32, tag="rstdT", name="rstdT")
                nc.vector.reciprocal(out=rstdT, in_=varT)
                rsp = ps_sm.tile([1, w], F32, tag="sm", name="lnrsp")
                for c in range(nch):
                    nc.tensor.matmul(rsp[0:1, c * P:(c + 1) * P], rstdT[:, c:c + 1],
                                     ident_f, start=True, stop=True)
                rstd = small.tile([1, w], F32, tag="rstd", name="rstd")
                nc.vector.tensor_copy(out=rstd, in_=rsp)
                meanB = ps_sm.tile([P, w], F32, tag="sm", name="lnmB")
                nc.tensor.matmul(meanB, onesrow_f, mean, start=True, stop=True)
                rstdB = ps_sm.tile([P, w], F32, tag="sm", name="lnrB")
                nc.tensor.matmul(rstdB, onesrow_f, rstd, start=True, stop=True)
                for k in range(6):
                    tmp = work.tile([P, w], F32, tag="lnt", name="lnt")
                    nc.vector.tensor_sub(out=tmp, in0=src_t[:, k, 0:w], in1=meanB)
                    nc.vector.tensor_mul(out=tmp, in0=tmp, in1=rstdB)
                    nc.vector.tensor_scalar(out=out_t[:, k, 0:w], in0=tmp,
                                            scalar1=g_sb[:, k:k+1],
                                            scalar2=b_sb[:, k:k+1],
                                            op0=mybir.AluOpType.mult,
                                            op1=mybir.AluOpType.add)

            # ================== preamble ==================
            load_early(0)
            load_late(0)
            load_small(0)
            prologue_kmems(0)
            prologue_rkT(0)
            for s, pieces in VSEG_PRO.items():
                v_seg(0, s, pieces)

            # ---- embedding: gather, transpose to d-major, scale ----
            gath = big1.tile([P, 4, D], F32, tag="big12")
            nc.gpsimd.dma_gather(out_ap=gath, in_ap=emb_d.ap(), idxs_ap=idxs,
                                 num_idxs=QLEN, num_idxs_reg=QLEN, elem_size=D)
            for it in range(4):
                for dt_ in range(6):
                    pt = ps_sm.tile([P, P], F32, tag="sm")
                    nc.tensor.transpose(pt, gath[:, it, dt_ * P:(dt_ + 1) * P], ident_f)
                    nc.scalar.mul(out=hT[:, dt_, it * P:(it + 1) * P], in_=pt,
                                  mul=float(np.sqrt(D)))

            # ============================ layers ============================
            for l in range(L):
                d = W[l]
                kT, rkT, v_tok = kT2[l % 2], rkT2[l % 2], vt2[l % 2]

                # ---- kT h-derived columns [544, 1056) ----
                for m in range(3):
                    pk = ps_sm.tile([P, QLEN], F32, tag="sm")
                    for k in range(6):
                        st, sp = (k == 0), (k == 5)
                        lhs = d['wk'][:, k, m * P:(m + 1) * P]
                        nc.tensor.matmul(pk[:, 0:480], lhs, hT[:, k, 0:480], start=st, stop=sp)
                        nc.tensor.matmul(pk[:, 480:512], lhs, hT[:, k, 480:512], start=st, stop=sp)
                    nc.scalar.copy(out=kT[:, m, MEMCOLS:KLEN], in_=pk)

                if l + 1 < L:
                    load_early(l + 1)

                # ---- q + rel biases ----
                for m in range(3):
                    pq = ps_sm.tile([P, QLEN], F32, tag="sm")
                    for k in range(6):
                        nc.tensor.matmul(pq, d['wq'][:, k, m * P:(m + 1) * P],
                                         hT[:, k, :], start=(k == 0), stop=(k == 5))
                    nc.vector.tensor_scalar_add(out=qrw[:, m, :], in0=pq,
                                                scalar1=rwb_t[:, m:m+1])
                    nc.vector.tensor_scalar_add(out=qrr[:, m, :], in0=pq,
                                                scalar1=rrb_t[:, m:m+1])

                # ---- v h-derived segments ----
                for s, pieces in VSEG_BODY.items():
                    v_seg(l, s, pieces)

                # ---- attention ----
                def bd_unit(n):
                    """BD raw (i-major) for all 4 i-tiles + skew writes."""
                    hp0 = 64 * (n % 2)
                    hk = n // 2
                    for t in range(4):
                        c0min = BDCH[t][0][0]
                        bd_i = work.tile([P, KLEN], BF16, tag="bdi", bufs=3)
                        lhs = qrr[hp0:hp0 + 64, hk, t * P:(t + 1) * P]
                        for ci, (c0, w) in enumerate(BDCH[t]):
                            pb = ps_sm.tile([P, 512], F32, tag="sm")
                            nc.tensor.matmul(pb[:, 0:w], lhs,
                                             rkT[hp0:hp0 + 64, hk, c0:c0 + w],
                                             start=True, stop=True)
                            if ci == 0:
                                nc.vector.tensor_copy(out=bd_i[:, c0:c0 + w], in_=pb[:, 0:w])
                            else:
                                nc.scalar.copy(out=bd_i[:, c0:c0 + w], in_=pb[:, 0:w])
                        dst = bass.AP(tensor=skew_d.ap().tensor,
                                      offset=n * QLEN * SKW + t * 128 * SKW + c0min,
                                      ap=[[SKW, 128], [1, KLEN - c0min]])
                        nc.gpsimd.dma_start(out=dst, in_=bd_i[:, c0min:])

                def score_av(n):
                    hp0 = 64 * (n % 2)
                    hk = n // 2
                    pav = ps_av.tile([VW, QLEN], F32, tag="pav")
                    for jt in range(8):
                        ist = ISTART[jt]
                        ni = QLEN - ist
                        # skew read through the xbar: lands transposed [j, i]
                        bdt = work.tile([P, QLEN], BF16, tag="bdt", bufs=3)
                        src = bass.AP(tensor=skew_d.ap().tensor,
                                      offset=n * QLEN * SKW + ist * (SKW - 1)
                                             + (QLEN - 1) + 128 * jt,
                                      ap=[[SKW - 1, ni], [1, 128]])
                        deng = nc.sync if jt % 2 == 0 else nc.scalar
                        deng.dma_start(out=bdt[:, ist:], in_=src, transpose=True)
                        pa = ps_pa.tile([P, QLEN], F32, tag="pa")
                        nc.tensor.matmul(pa[:, ist:], kT[hp0:hp0 + 64, hk, 128 * jt:128 * jt + 128],
                                         qrw[hp0:hp0 + 64, hk, ist:],
                                         start=True, stop=True)
                        sc = work.tile([P, QLEN], F32, tag="sc", bufs=3)
                        nc.vector.tensor_add(out=sc[:, ist:], in0=pa[:, ist:],
                                             in1=bdt[:, ist:])
                        ex = work.tile([P, QLEN], BF16, tag="ex", bufs=3)
                        nc.scalar.activation(out=ex[:, ist:], in_=sc[:, ist:],
                                             func=mybir.ActivationFunctionType.Exp,
                                             bias=0.0, scale=SCALE)
                        nc.tensor.matmul(pav[:, ist:],
                                         v_tok[:, jt, VW * n:VW * n + VW],
                                         ex[:, ist:],
                                         start=(jt == 0), stop=False)
                    # corner: j in [1024, 1056), i in [480, 512)
                    bds8 = smalls.tile([32, 32], BF16, tag="bds8", name="bds8")
                    src = bass.AP(tensor=skew_d.ap().tensor,
                                  offset=n * QLEN * SKW + 480 * (SKW - 1)
                                         + (QLEN - 1) + 1024,
                                  ap=[[SKW - 1, 32], [1, 32]])
                    nc.scalar.dma_start(out=bds8, in_=src)
                    pst = ps_sm.tile([32, 32], BF16, tag="smt", bufs=1)
                    nc.tensor.transpose(pst, bds8, ident_b[0:32, 0:32])
                    sb8 = smalls.tile([32, 32], BF16, tag="sb8", name="sb8")
                    nc.scalar.copy(out=sb8, in_=pst)
                    pa8 = ps_pa.tile([P, QLEN], F32, tag="pa")
                    nc.tensor.matmul(pa8[0:32, 0:32], kT[hp0:hp0 + 64, hk, 1024:1056],
                                     qrw[hp0:hp0 + 64, hk, 480:512],
                                     start=True, stop=True)
                    sc8 = smalls.tile([32, 32], F32, tag="sc8", name="sc8")
                    nc.vector.tensor_add(out=sc8, in0=pa8[0:32, 0:32], in1=sb8)
                    ex8 = smalls.tile([32, 32], BF16, tag="ex8", name="ex8")
                    nc.scalar.activation(out=ex8, in_=sc8,
                                         func=mybir.ActivationFunctionType.Exp,
                                         bias=0.0, scale=SCALE)
                    nc.tensor.matmul(pav[:, 480:512],
                                     v_tok[0:32, 8, VW * n:VW * n + VW],
                                     ex8, start=False, stop=True)
                    # normalize: row 64 of pav is the denominator
                    rd = smalls.tile([1, QLEN], F32, tag="rd", name="rd")
                    nc.vector.reciprocal(out=rd, in_=pav[DH:DH + 1, :])
                    rdB_ps = ps_sm.tile([P, 512], F32, tag="sm")
                    nc.tensor.matmul(rdB_ps[0:DH, :], onesrow_f[0:1, 0:DH], rd,
                                     start=True, stop=True)
                    rdB = work.tile([DH, QLEN], F32, tag="rdB", name="rdB")
                    nc.scalar.copy(out=rdB, in_=rdB_ps[0:DH, :])
                    nc.vector.tensor_mul(out=av_sb[hp0:hp0 + 64, hk, :],
                                         in0=pav[0:DH, :], in1=rdB)

                for n in range(NH_L):
                    bd_unit(n)
                    if n >= 1:
                        score_av(n - 1)
                score_av(NH_L - 1)

                # ---- o-proj + residual/2 -> ReduceScatter ----
                for m in range(6):
                    po = ps_sm.tile([P, QLEN], F32, tag="sm", name="po")
                    for k in range(3):
                        nc.tensor.matmul(po, d['ow'][:, k, m * P:(m + 1) * P],
                                         av_sb[:, k, :],
                                         start=(k == 0), stop=(k == 2))
                    ob = work.tile([P, QLEN], BF16, tag="ob", name="ob")
                    nc.vector.scalar_tensor_tensor(
                        out=ob, in0=hT[:, m, :], scalar=0.5,
                        in1=po, op0=mybir.AluOpType.mult,
                        op1=mybir.AluOpType.add)
                    dst = bass.AP(tensor=cc_rs_in.ap().tensor,
                                  offset=m * P * H2,
                                  ap=[[H2, 128], [D * H2, 2], [1, H2]])
                    nc.gpsimd.dma_start(out=dst, in_=ob[:, :].rearrange("p (s t) -> p s t", s=2))
                nc.gpsimd.collective_compute(
                    "ReduceScatter", mybir.AluOpType.add, replica_groups=RG,
                    ins=[cc_rs_in.ap()], outs=[cc_rs_out.ap()])

                if l + 1 < L:
                    load_late(l + 1)
                    # ---- fill RS stall: next layer's kT mems + v mems ----
                    prologue_kmems(l + 1)
                    for s, pieces in VSEG_PRO.items():
                        v_seg(l + 1, s, pieces)

                # ---- RS readback -> LN1 -> FFN ----
                nc.gpsimd.dma_start(
                    out=own_pre,
                    in_=cc_rs_out.ap().rearrange("(k p) t -> p k t", p=P))
                ln_dmajor(own_pre, d['ln1g'], d['ln1b'], h1)

                ffn1 = big1.tile([P, 24, H2], BF16, tag="ffn1")
                for km in range(24):
                    pf = ps_sm.tile([P, H2], F32, tag="sm", name="pf")
                    wsl = wstream.tile([P, 6, P], BF16, tag="w1s", name="w1s", bufs=3)
                    src = bass.AP(tensor=w1_d.ap().tensor,
                                  offset=l * D * DI + km * P,
                                  ap=[[DI, P], [P * DI, 6], [1, P]])
                    nc.sync.dma_start(out=wsl, in_=src)
                    for k in range(6):
                        nc.tensor.matmul(pf, wsl[:, k, :], h1[:, k, :],
                                         start=(k == 0), stop=(k == 5))
                    nc.scalar.activation(out=ffn1[:, km, :], in_=pf,
                                         func=mybir.ActivationFunctionType.Relu,
                                         bias=d['b1'][:, km:km+1], scale=1.0)

                for m in range(6):
                    pf = ps_sm.tile([P, H2], F32, tag="sm", name="pf2")
                    for g in range(2):
                        wsl = wstream.tile([P, 12, P], BF16, tag="w2s", name="w2s")
                        src = bass.AP(tensor=w2_d.ap().tensor,
                                      offset=l * DI * D + g * 12 * P * D + m * P,
                                      ap=[[D, P], [P * D, 12], [1, P]])
                        nc.sync.dma_start(out=wsl, in_=src)
                        for k in range(12):
                            nc.tensor.matmul(pf, wsl[:, k, :], ffn1[:, g * 12 + k, :],
                                             start=(g == 0 and k == 0),
                                             stop=(g == 1 and k == 11))
                    fb = work.tile([P, H2], F32R, tag="fb", name="fb")
                    nc.vector.tensor_scalar_add(out=fb, in0=pf,
                                                scalar1=d['b2'][:, m:m+1])
                    nc.vector.tensor_add(out=h2pre[:, m, :], in0=fb, in1=h1[:, m, :])

                ln_dmajor(h2pre, d['ln2g'], d['ln2b'], h_new)

                if l + 1 < L:
                    # ---- AllGather h_new ----
                    nc.gpsimd.dma_start(
                        out=cc_ag_in.ap().rearrange("(k p) t -> p k t", p=P),
                        in_=h_new)
                    nc.gpsimd.collective_compute(
                        "AllGather", mybir.AluOpType.bypass, replica_groups=RG,
                        ins=[cc_ag_in.ap()], outs=[cc_ag_out.ap()])
                    # ---- fill AG stall: next layer's rkT ----
                    prologue_rkT(l + 1)
                    # ---- AG readback -> full hT ----
                    for sseg in range(2):
                        nc.sync.dma_start(
                            out=hT[:, :, sseg * H2:(sseg + 1) * H2],
                            in_=cc_ag_out.ap()[sseg].rearrange("(k p) t -> p k t", p=P))
                    load_small(l + 1)

            # ---- final projection: own 256 tokens x full vocab ----
            NCH = 500
            for c in range(V // NCH):
                wsl = pstream.tile([P, 6, NCH], BF16, tag="pws")
                src = bass.AP(tensor=projw_d.ap().tensor,
                              offset=c * NCH,
                              ap=[[V, P], [P * V, 6], [1, NCH]])
                nc.sync.dma_start(out=wsl, in_=src)
                pbs = small.tile([1, NCH], F32R, tag="pbs", bufs=2)
                nc.gpsimd.dma_start(out=pbs, in_=projb_d.ap()[0:1, c * NCH:(c + 1) * NCH])
                for tt in range(2):
                    pp = ps_sm.tile([P, NCH], F32, tag="sm")
                    for k in range(6):
                        nc.tensor.matmul(pp, h_new[:, k, tt * P:(tt + 1) * P],
                                         wsl[:, k, :], start=(k == 0), stop=False)
                    nc.tensor.matmul(pp, onesrow_r, pbs, start=False, stop=True)
                    osb = work.tile([P, NCH], F32, tag="osb")
                    nc.vector.tensor_copy(out=osb, in_=pp)
                    nc.sync.dma_start(out=out_d.ap()[tt * P:(tt + 1) * P,
                                                     c * NCH:(c + 1) * NCH],
                                      in_=osb)

    nc.compile()
    return nc


def _pos_emb_T(klen):
    pos = np.arange(klen - 1, -1, -1, dtype=np.float32)
    inv = 1.0 / (10000.0 ** (np.arange(0, D, 2, dtype=np.float32) / D))
    s = pos[:, None] * inv[None, :]
    r = np.concatenate([np.sin(s), np.cos(s)], axis=-1)
    return np.ascontiguousarray(r.T)  # [D, klen]


def kernel(x, condition, mems, emb, qkv_w, r_net_w, o_w, ln1_g, ln1_b,
           w1, b1, w2, b2, ln2_g, ln2_b, r_w_bias, r_r_bias, proj_w, proj_b):
    import ml_dtypes
    BF = ml_dtypes.bfloat16

    L = int(os.environ.get("KERNEL_LAYERS", str(L_FULL)))
    if L not in _BUILD_CACHE:
        _BUILD_CACHE[L] = _build(L)
    nc = _BUILD_CACHE[L]

    f32 = lambda a: np.asarray(a, dtype=np.float32)
    bf = lambda a: np.ascontiguousarray(np.asarray(a, dtype=np.float32).astype(BF))
    x = np.asarray(x)
    condition = f32(condition); mems = f32(mems); emb = f32(emb)
    qkv_w = f32(qkv_w); r_net_w = f32(r_net_w); o_w = f32(o_w)
    ln1_g = f32(ln1_g); ln1_b = f32(ln1_b); w1 = f32(w1); b1 = f32(b1)
    w2 = f32(w2); b2 = f32(b2); ln2_g = f32(ln2_g); ln2_b = f32(ln2_b)
    r_w_bias = f32(r_w_bias); r_r_bias = f32(r_r_bias)
    proj_w = f32(proj_w); proj_b = f32(proj_b)

    rT = _pos_emb_T(KLEN)

    in_maps = []
    for c in range(N_CORES):
        b, half = c // TP, c % TP
        toks = np.asarray(x[:, b], dtype=np.int64)
        idxw = np.zeros((128, 32), np.int16)
        ar = toks.reshape(32, 16).astype(np.int16)  # token i = col*16 + row
        for k in range(8):
            idxw[16 * k:16 * (k + 1), :] = ar.T
        hs = slice(half * HD_L, (half + 1) * HD_L)
        m = {
            "emb": np.ascontiguousarray(emb),
            "idx": idxw,
            "condT": bf(condition[:, b, :].T),
            "memsT": bf(mems[:L, :, b, :].transpose(0, 2, 1)),
            "rT": bf(rT),
            "wq": bf(qkv_w[:L, :, hs]),
            "wk": bf(qkv_w[:L, :, 768 + half * HD_L:768 + (half + 1) * HD_L]),
            "wv": bf(qkv_w[:L, :, 1536 + half * HD_L:1536 + (half + 1) * HD_L]),
            "rnet": bf(r_net_w[:L, :, hs]),
            "ow": bf(o_w[:L, hs, :]),
            "w1": bf(w1[:L]),
            "b1": np.ascontiguousarray(b1[:L]).reshape(L, 24, 128),
            "w2": bf(w2[:L]),
            "b2": np.ascontiguousarray(b2[:L]).reshape(L, 6, 128),
            "ln1g": np.ascontiguousarray(ln1_g[:L]).reshape(L, 6, 128),
            "ln1b": np.ascontiguousarray(ln1_b[:L]).reshape(L, 6, 128),
            "ln2g": np.ascontiguousarray(ln2_g[:L]).reshape(L, 6, 128),
            "ln2b": np.ascontiguousarray(ln2_b[:L]).reshape(L, 6, 128),
            "rwb": np.ascontiguousarray(r_w_bias.reshape(NH * DH)[half * HD_L:(half + 1) * HD_L]).reshape(3, 128),
            "rrb": np.ascontiguousarray(r_r_bias.reshape(NH * DH)[half * HD_L:(half + 1) * HD_L]).reshape(3, 128),
            "projw": bf(proj_w),
            "projb": np.ascontiguousarray(proj_b).reshape(1, V),
        }
        in_maps.append(m)

    trace = bool(int(os.environ.get("KERNEL_TRACE", "0")))
    res = run_bass_kernel_spmd(nc, in_maps, core_ids=list(range(N_CORES)),
                               trace=trace)
    kernel.last_result = res

    out = np.zeros((QLEN, BSZ, V), np.float32)
    for c in range(N_CORES):
        b, half = c // TP, c % TP
        out[half * H2:(half + 1) * H2, b, :] = res.results[c]["logits"]
    return out
